# revision 32
# baseline (speedup 1.0000x reference)
"""Trainium2 Bass kernel for nn_Attention_41729902248209.

8-head attention block: x (8, 512, 32, 32) -> QKV proj -> softmax attention
-> out proj + residual. Data-parallel over batch: one batch element per
NeuronCore (8 cores).

Per-core dataflow (n = 1024 tokens, cin = 512, H = 8 heads, D = 64):
  - everything stays "transposed" (feature dim on partitions) so no on-chip
    transposes are needed anywhere:
      qT, kT : (f' = 64h+d on partitions, n free) fp16 [head pairs share tiles]
      v8     : (n on partitions, 66h+e free, fp8e4) with a ones column per
               head at e=64 (integrates the softmax denominator in attnv)
      scoresT: (j on partitions, i free) = k @ qT   [2 heads via tile_position]
      pT     : exp(scoresT - 3) in fp8e4 straight off PSUM (the -3 keeps exp
               under the TRN fp8e4 240-max normal; cancels in softmax)
      outT~  : [v8 | 1].T @ pT8 -> (65, i) in PSUM: rows 0:64 = unscaled outT,
               row 64 = softmax denominator
      os8    : outT * (1/denom) in fp8e4
      yT     : wl8.T @ os8 + (x + b') residual, fp32
  - biases: b_q/b_k fused into the qk evac (DVE tensor_scalar_add); b_v is
    folded host-side into the residual (softmax weights sum to 1, so
    Sum p (v+bv) = Sum p v + bv -> b' = b_last + W_last @ b_v); softmax
    scale 1/8 folded into W_q host-side.
  - denominators (per (head, chunk)): one DVE reciprocal straight off the
    PSUM denom row (partition 64) writing partition 0, GPSIMD
    partition_broadcast, then one DVE multiply off PSUM into os8 (fp8).
  - emission is software-pipelined at single-score-tile granularity: the
    attnv work is a stream of 2-matmul units drained behind the scores/exp
    stream so ACT (the ~66 us exp floor) never starves while PE stays dense.
"""

import numpy as np
import ml_dtypes

import concourse.mybir as mybir
import concourse.tile as tile
from concourse import bacc
from concourse.bass_utils import run_bass_kernel_spmd

F8 = mybir.dt.float8e4
F16 = mybir.dt.float16
F32 = mybir.dt.float32

BS = 8
H = 8
D = 64
CIN = 512
N = 1024
NK = CIN // 128  # contraction tiles for cin
NJT = N // 128  # j tiles
NCH = N // 512  # i chunks of 512
VR = D + 2  # 66: per-head v block [v_h (64) | 1 | pad]
VROW = H * VR  # 528 per j-tile
EXPC = 3.0  # exp shift: p = exp(s - EXPC); cancels in softmax

AF = mybir.ActivationFunctionType
ALU = mybir.AluOpType


def _emit(tc, d, sb, ps):
    nc = tc.nc

    x16_sb = sb.tile([128, NK * N], F16, tag="x16")
    xr_sb = sb.tile([128, NK * N], F32, tag="xr")
    wq_sb = sb.tile([128, NK * 512], F16, tag="wq")
    wk_sb = sb.tile([128, NK * 512], F16, tag="wk")
    wv_sb = sb.tile([128, NK * 512], F16, tag="wv")
    wl8_sb = sb.tile([128, NK * 512], F8, tag="wl8")
    bqk_sb = sb.tile([128, 8], F32, tag="bqk")
    expb_sb = sb.tile([128, 1], F32, tag="expb")
    qT_sb = sb.tile([128, 4 * N], F16, tag="qT")
    kT_sb = sb.tile([128, 4 * N], F16, tag="kT")
    v8_sb = sb.tile([128, NJT * VROW], F8, tag="v8")
    os_sb = sb.tile([128, NK * N], F8, tag="os8")

    # --- input DMAs (ktile k of a (512, W) dram tensor -> cols [W*k, W*k+W))
    # Issue is the bottleneck (one sequencer = ~0.65us per DMA, serial), so
    # spread the loads across idle engines' DGE queues; bqk rides first on
    # sync (it gates the first qk evac).
    nc.sync.dma_start(bqk_sb[:], d["bqk"].ap())
    for k in range(NK):
        r = slice(128 * k, 128 * k + 128)
        nc.sync.dma_start(wq_sb[:, 512 * k : 512 * k + 512], d["wq"].ap()[r, :])
        nc.gpsimd.dma_start(x16_sb[:, N * k : N * k + N], d["x16"].ap()[r, :])
        nc.scalar.dma_start(wk_sb[:, 512 * k : 512 * k + 512], d["wk"].ap()[r, :])
    for k in range(NK):
        r = slice(128 * k, 128 * k + 128)
        nc.sync.dma_start(wv_sb[:, 512 * k : 512 * k + 512], d["wv"].ap()[r, :])
    nc.vector.memset(expb_sb[:], -EXPC)
    # ones columns for v8 (column 64 of each 66-wide head block)
    v8_blocks = v8_sb[:].rearrange("p (jt h e) -> p jt h e", jt=NJT, e=VR)
    nc.vector.memset(v8_blocks[:, :, :, D : D + 1], 1.0)
    for k in range(NK):
        r = slice(128 * k, 128 * k + 128)
        nc.scalar.dma_start(wl8_sb[:, 512 * k : 512 * k + 512], d["wl8"].ap()[r, :])
        nc.gpsimd.dma_start(xr_sb[:, N * k : N * k + N], d["xr"].ap()[r, :])

    # --- stage emitters ---
    def qk_tile(t):
        """Project q and k for f'-tile t (heads 2t, 2t+1), with bias."""
        for wsb, dst, bcol in ((wq_sb, qT_sb, t), (wk_sb, kT_sb, 4 + t)):
            for c in range(NCH):
                p = ps.tile([128, 512], F32, tag="mm")
                for k in range(NK):
                    nc.tensor.matmul(
                        p[:],
                        wsb[:, 512 * k + 128 * t : 512 * k + 128 * t + 128],
                        x16_sb[:, N * k + 512 * c : N * k + 512 * c + 512],
                        start=(k == 0),
                        stop=(k == NK - 1),
                    )
                nc.vector.tensor_scalar_add(
                    dst[:, N * t + 512 * c : N * t + 512 * c + 512],
                    p[:],
                    bqk_sb[:, bcol : bcol + 1],
                )

    def v_tile(jt):
        """Project v for token tile jt: (128 tokens, 512 feats) -> v8 fp8."""
        p = ps.tile([128, 512], F32, tag="mm")
        for k in range(NK):
            nc.tensor.matmul(
                p[:],
                x16_sb[:, N * k + 128 * jt : N * k + 128 * jt + 128],
                wv_sb[:, 512 * k : 512 * k + 512],
                start=(k == 0),
                stop=(k == NK - 1),
            )
        nc.vector.tensor_copy(
            v8_blocks[:, jt, :, 0:D],
            p[:].rearrange("p (h e) -> p h e", e=D),
        )

    pt_tiles = {}

    def scores_exp(h, jts):
        """scoresT (j, i) for head h + exp(s - 3) -> pT fp8e4."""
        pr, hh = divmod(h, 2)
        if h in pt_tiles:
            pT = pt_tiles[h]
        else:
            pT = sbuf_pt_pool.tile([128, NJT * N], F8, tag="pt", name=f"pt{h}")
            pt_tiles[h] = pT
        po = 64 * hh
        for jt in jts:
            sp = ps.tile([128, N], F32, tag="score", bufs=2)
            for c in range(NCH):
                nc.tensor.matmul(
                    sp[:, 512 * c : 512 * c + 512],
                    kT_sb[po : po + 64, N * pr + 128 * jt : N * pr + 128 * jt + 128],
                    qT_sb[po : po + 64, N * pr + 512 * c : N * pr + 512 * c + 512],
                    start=True,
                    stop=True,
                    tile_position=(po, 0),
                )
            nc.scalar.activation(
                pT[:, N * jt : N * jt + N], sp[:], AF.Exp, bias=expb_sb[:]
            )

    pv_tiles = {}

    def attnv_unit(h, c, g2):
        """2 accumulating fp8 matmuls (j-tiles 2*g2, 2*g2+1) of outT~ for
        (h, c); denominator + os8 scaling after the last unit of the chunk."""
        pr, hh = divmod(h, 2)
        pT = pt_tiles[h]
        key = (h, c)
        if key not in pv_tiles:
            pv_tiles[key] = ps.tile([128, 512], F32, tag="mm", name=f"av{h}_{c}")
        p = pv_tiles[key]
        for jt in (2 * g2, 2 * g2 + 1):
            nc.tensor.matmul(
                p[0:65, :],
                v8_blocks[:, jt, h, 0 : D + 1],
                pT[:, N * jt + 512 * c : N * jt + 512 * c + 512],
                start=(jt == 0),
                stop=(jt == NJT - 1),
            )
        if g2 == 3:
            del pv_tiles[key]
            if c == NCH - 1:
                del pt_tiles[h]
            r = 2 * h + c
            # denominator chain, DMA-free: one reciprocal straight off the
            # PSUM denom row (p64) writing partition 0, gpsimd broadcast,
            # then one fp32 multiply straight off PSUM into os8 (fp8).
            rrow = rr_pool.tile([1, 512], F32, tag="rrow", name=f"rr{r}")
            nc.vector.reciprocal(rrow[0:1, :], p[64:65, :])
            rb = rb_pool.tile([128, 512], F32, tag="rb", name=f"rb{r}")
            nc.gpsimd.partition_broadcast(rb[:], rrow[0:1, :])
            sl = slice(N * pr + 512 * c, N * pr + 512 * c + 512)
            nc.vector.tensor_tensor(
                os_sb[64 * hh : 64 * hh + 64, sl],
                p[0:64, :],
                rb[0:64, :],
                ALU.mult,
            )

    def outproj(ct):
        """yT c-tile ct: wl8.T @ os8 (fp8) + (x + b'), fp32 out + DMA.

        PSUM comes from the score tag's banks (idle once exps are done)."""
        p = ps.tile([128, N], F32, tag="score", bufs=2, name=f"yp{ct}")
        for c in range(NCH):
            for k in range(NK):
                nc.tensor.matmul(
                    p[:, 512 * c : 512 * c + 512],
                    wl8_sb[:, 512 * k + 128 * ct : 512 * k + 128 * ct + 128],
                    os_sb[:, N * k + 512 * c : N * k + 512 * c + 512],
                    start=(k == 0),
                    stop=(k == NK - 1),
                )
        if ct < 3:
            y = y_pool.tile([128, N], F32, tag="y")
            nc.vector.tensor_tensor(
                y[:], p[:], xr_sb[:, N * ct : N * ct + N], ALU.add
            )
            nc.sync.dma_start(d["y"].ap()[128 * ct : 128 * ct + 128, :], y[:])
        else:
            # split the last c-tile so its evac/DMA pipeline drains earlier
            for c in range(NCH):
                sl = slice(512 * c, 512 * c + 512)
                y = y_pool.tile([128, 512], F32, tag="y2", name=f"y3_{c}")
                nc.vector.tensor_tensor(
                    y[:], p[:, sl], xr_sb[:, N * ct + 512 * c : N * ct + 512 * c + 512],
                    ALU.add,
                )
                nc.sync.dma_start(
                    d["y"].ap()[128 * ct : 128 * ct + 128, sl], y[:]
                )

    # --- pools that emitters close over ---
    import contextlib

    stack = contextlib.ExitStack()
    sbuf_pt_pool = stack.enter_context(tc.tile_pool(name="pt", bufs=3))
    rb_pool = stack.enter_context(tc.tile_pool(name="rb", bufs=3))
    rr_pool = stack.enter_context(tc.tile_pool(name="rr", bufs=3))
    y_pool = stack.enter_context(tc.tile_pool(name="y", bufs=3))

    # --- software-pipelined emission (PE order: keep feeding while ACT exps) ---
    av_units = [(h, c, g2) for h in range(H) for c in range(NCH) for g2 in range(4)]
    av_pos = 0

    def drain_av(n):
        nonlocal av_pos
        for _ in range(n):
            if av_pos >= len(av_units):
                return
            h, c, g2 = av_units[av_pos]
            av_pos += 1
            attnv_unit(h, c, g2)

    qk_tile(0)
    qk_tile(1)
    qk_quarters = [
        (wsb, dst, bcol, t, c)
        for t in (2, 3)
        for (wsb, dst, bcol) in ((wq_sb, qT_sb, t), (wk_sb, kT_sb, 4 + t))
        for c in range(NCH)
    ]
    for g in range(NJT):  # head 0 scores + qk tiles 2,3 (one quarter per step)
        scores_exp(0, [g])
        wsb, dst, bcol, t, c = qk_quarters[g]
        p = ps.tile([128, 512], F32, tag="mm", name=f"qk{t}_{bcol}_{c}")
        for k in range(NK):
            nc.tensor.matmul(
                p[:],
                wsb[:, 512 * k + 128 * t : 512 * k + 128 * t + 128],
                x16_sb[:, N * k + 512 * c : N * k + 512 * c + 512],
                start=(k == 0),
                stop=(k == NK - 1),
            )
        nc.vector.tensor_scalar_add(
            dst[:, N * t + 512 * c : N * t + 512 * c + 512],
            p[:],
            bqk_sb[:, bcol : bcol + 1],
        )
    for g in range(NJT):  # head 1 scores + v tiles
        scores_exp(1, [g])
        v_tile(g)
    rates = {2: 8, 3: 8, 4: 8, 5: 8, 6: 8, 7: 8}
    for h in range(2, H):
        per = [rates[h] // NJT + (1 if g < rates[h] % NJT else 0) for g in range(NJT)]
        for g in range(NJT):
            scores_exp(h, [g])
            drain_av(per[g])
    drain_av(len(av_units))  # remainder (attnv of heads 6,7 + last denoms)
    for ct in range(4):
        outproj(ct)

    stack.close()


def _build(loop=1):
    nc = bacc.Bacc("TRN2", target_bir_lowering=False, debug=False, num_devices=BS)
    d = {}
    d["x16"] = nc.dram_tensor("x16", [CIN, N], F16, kind="ExternalInput")
    d["xr"] = nc.dram_tensor("xr", [CIN, N], F32, kind="ExternalInput")
    d["wq"] = nc.dram_tensor("wq", [CIN, 512], F16, kind="ExternalInput")
    d["wk"] = nc.dram_tensor("wk", [CIN, 512], F16, kind="ExternalInput")
    d["wv"] = nc.dram_tensor("wv", [CIN, 512], F16, kind="ExternalInput")
    d["wl8"] = nc.dram_tensor("wl8", [CIN, 512], F8, kind="ExternalInput")
    d["bqk"] = nc.dram_tensor("bqk", [128, 8], F32, kind="ExternalInput")
    d["y"] = nc.dram_tensor("y", [CIN, N], F32, kind="ExternalOutput")

    with tile.TileContext(nc) as tc:
        with (
            tc.tile_pool(name="sb", bufs=1) as sb,
            tc.tile_pool(name="ps", bufs=4, space="PSUM") as ps,
        ):
            for i in range(loop):
                if i:
                    with tc.tile_critical():
                        nc.all_engine_barrier()
                _emit(tc, d, sb, ps)
    nc.compile()
    return nc


_NC_CACHE = {}


def get_nc(loop=1):
    if loop not in _NC_CACHE:
        _NC_CACHE[loop] = _build(loop)
    return _NC_CACHE[loop]


def host_prep(x, W_fc, b_fc, W_last, b_last):
    """Full inputs -> list of 8 per-core input maps."""
    x = np.asarray(x, dtype=np.float64)
    W_fc = np.asarray(W_fc, dtype=np.float64)
    b_fc = np.asarray(b_fc, dtype=np.float64)
    W_last = np.asarray(W_last, dtype=np.float64)
    b_last = np.asarray(b_last, dtype=np.float64)

    hh = np.arange(H).repeat(D) * 3 * D  # 192h per f'=64h+d
    dd = np.tile(np.arange(D), H)
    pq, pk, pv = hh + dd, hh + D + dd, hh + 2 * D + dd

    wq = np.ascontiguousarray((W_fc[pq] * 0.125).T).astype(np.float16)
    wk = np.ascontiguousarray(W_fc[pk].T).astype(np.float16)
    wv = np.ascontiguousarray(W_fc[pv].T).astype(np.float16)
    wl8 = np.ascontiguousarray(W_last.T).astype(ml_dtypes.float8_e4m3)
    bq, bk, bv = b_fc[pq] * 0.125, b_fc[pk], b_fc[pv]
    bqk = np.ascontiguousarray(
        np.concatenate([bq.reshape(4, 128).T, bk.reshape(4, 128).T], axis=1)
    ).astype(np.float32)
    # b_v passes through softmax unchanged (weights sum to 1): fold W_last@bv
    # into the residual bias.
    b_eff = b_last + W_last @ bv

    xf = x.reshape(BS, CIN, N)
    maps = []
    for b in range(BS):
        maps.append(
            {
                "x16": xf[b].astype(np.float16),
                "xr": (xf[b] + b_eff[:, None]).astype(np.float32),
                "wq": wq,
                "wk": wk,
                "wv": wv,
                "wl8": wl8,
                "bqk": bqk,
            }
        )
    return maps


def kernel(x, W_fc, b_fc, W_last, b_last):
    nc = get_nc()
    maps = host_prep(x, W_fc, b_fc, W_last, b_last)
    res = run_bass_kernel_spmd(nc, maps, core_ids=list(range(BS)))
    y = np.stack([res.results[b]["y"] for b in range(BS)])
    return y.reshape(BS, CIN, 32, 32)


# revision 33
# speedup vs baseline: 1.4589x; 1.4589x over previous
"""Trainium2 Bass kernel for nn_Attention_41729902248209.

8-head attention block: x (8, 512, 32, 32) -> QKV proj -> softmax attention
-> out proj + residual. Data-parallel over batch: one batch element per
NeuronCore (8 cores).

Per-core dataflow (n = 1024 tokens, cin = 512, H = 8 heads, D = 64):
  - everything stays "transposed" (feature dim on partitions) so no on-chip
    transposes are needed anywhere:
      qT, kT : (f' = 64h+d on partitions, n free) fp16 [head pairs share tiles]
      v8     : (n on partitions, 66h+e free, fp8e4) with a ones column per
               head at e=64 (integrates the softmax denominator in attnv)
      scoresT: (j on partitions, i free) = k @ qT   [2 heads via tile_position]
      pT     : exp(scoresT - 3) in fp8e4 straight off PSUM (the -3 keeps exp
               under the TRN fp8e4 240-max normal; cancels in softmax)
      outT~  : [v8 | 1].T @ pT8 -> (65, i) in PSUM: rows 0:64 = unscaled outT,
               row 64 = softmax denominator
      os8    : outT * (1/denom) in fp8e4
      yT     : wl8.T @ os8 + (x + b') residual, fp32
  - biases: b_q/b_k fused into the qk evac (DVE tensor_scalar_add); b_v is
    folded host-side into the residual (softmax weights sum to 1, so
    Sum p (v+bv) = Sum p v + bv -> b' = b_last + W_last @ b_v); softmax
    scale 1/8 folded into W_q host-side.
  - denominators (per (head, chunk)): one DVE reciprocal straight off the
    PSUM denom row (partition 64) writing partition 0, GPSIMD
    partition_broadcast, then one DVE multiply off PSUM into os8 (fp8).
  - emission is software-pipelined at single-score-tile granularity: the
    attnv work is a stream of 2-matmul units drained behind the scores/exp
    stream so ACT (the ~66 us exp floor) never starves while PE stays dense.
"""

import numpy as np
import ml_dtypes

import concourse.mybir as mybir
import concourse.tile as tile
from concourse import bacc
from concourse.bass_utils import run_bass_kernel_spmd

F8 = mybir.dt.float8e4
F16 = mybir.dt.float16
F32 = mybir.dt.float32

BS = 8
H = 8
D = 64
CIN = 512
N = 1024
NK = CIN // 128  # contraction tiles for cin
NJT = N // 128  # j tiles
NCH = N // 512  # i chunks of 512
VR = D + 2  # 66: per-head v block [v_h (64) | 1 | pad]
VROW = H * VR  # 528 per j-tile
EXPC = 3.0  # exp shift: p = exp(s - EXPC); cancels in softmax

AF = mybir.ActivationFunctionType
ALU = mybir.AluOpType
PM = mybir.MatmulPerfMode

USE_DR_ATTNV = False  # fp8 DoubleRow attnv (j-tile pairs)
USE_DR_OUTPROJ = False  # fp8 DoubleRow outproj (k-tile pairs)


def _emit(tc, d, sb, ps):
    nc = tc.nc

    x16_sb = sb.tile([128, NK * N], F16, tag="x16")
    xr_sb = sb.tile([128, NK * N], F32, tag="xr")
    wq_sb = sb.tile([128, NK * 512], F16, tag="wq")
    wk_sb = sb.tile([128, NK * 512], F16, tag="wk")
    wv_sb = sb.tile([128, NK * 512], F16, tag="wv")
    wl8_sb = sb.tile([128, NK * 512], F8, tag="wl8")
    bqk_sb = sb.tile([128, 8], F32, tag="bqk")
    expb_sb = sb.tile([128, 1], F32, tag="expb")
    qT_sb = sb.tile([128, 4 * N], F16, tag="qT")
    kT_sb = sb.tile([128, 4 * N], F16, tag="kT")
    v8_sb = sb.tile([128, NJT * VROW], F8, tag="v8")
    os_sb = sb.tile([128, NK * N], F8, tag="os8")

    # --- input DMAs (ktile k of a (512, W) dram tensor -> cols [W*k, W*k+W))
    # Issue is the bottleneck (one sequencer = ~0.65us per DMA, serial), so
    # spread the loads across idle engines' DGE queues; bqk rides first on
    # sync (it gates the first qk evac).
    nc.sync.dma_start(bqk_sb[:], d["bqk"].ap())
    for k in range(NK):
        r = slice(128 * k, 128 * k + 128)
        nc.sync.dma_start(wq_sb[:, 512 * k : 512 * k + 512], d["wq"].ap()[r, :])
        nc.gpsimd.dma_start(x16_sb[:, N * k : N * k + N], d["x16"].ap()[r, :])
        nc.scalar.dma_start(wk_sb[:, 512 * k : 512 * k + 512], d["wk"].ap()[r, :])
    for k in range(NK):
        r = slice(128 * k, 128 * k + 128)
        nc.sync.dma_start(wv_sb[:, 512 * k : 512 * k + 512], d["wv"].ap()[r, :])
    nc.vector.memset(expb_sb[:], -EXPC)
    # ones columns for v8 (column 64 of each 66-wide head block)
    v8_blocks = v8_sb[:].rearrange("p (jt h e) -> p jt h e", jt=NJT, e=VR)
    nc.vector.memset(v8_blocks[:, :, :, D : D + 1], 1.0)
    for k in range(NK):
        r = slice(128 * k, 128 * k + 128)
        nc.scalar.dma_start(wl8_sb[:, 512 * k : 512 * k + 512], d["wl8"].ap()[r, :])
        nc.gpsimd.dma_start(xr_sb[:, N * k : N * k + N], d["xr"].ap()[r, :])

    # --- stage emitters ---
    def qk_tile(t):
        """Project q and k for f'-tile t (heads 2t, 2t+1), with bias."""
        for wsb, dst, bcol in ((wq_sb, qT_sb, t), (wk_sb, kT_sb, 4 + t)):
            for c in range(NCH):
                p = ps.tile([128, 512], F32, tag="mm")
                for k in range(NK):
                    nc.tensor.matmul(
                        p[:],
                        wsb[:, 512 * k + 128 * t : 512 * k + 128 * t + 128],
                        x16_sb[:, N * k + 512 * c : N * k + 512 * c + 512],
                        start=(k == 0),
                        stop=(k == NK - 1),
                    )
                nc.vector.tensor_scalar_add(
                    dst[:, N * t + 512 * c : N * t + 512 * c + 512],
                    p[:],
                    bqk_sb[:, bcol : bcol + 1],
                )

    def v_tile(jt):
        """Project v for token tile jt: (128 tokens, 512 feats) -> v8 fp8."""
        p = ps.tile([128, 512], F32, tag="mm")
        for k in range(NK):
            nc.tensor.matmul(
                p[:],
                x16_sb[:, N * k + 128 * jt : N * k + 128 * jt + 128],
                wv_sb[:, 512 * k : 512 * k + 512],
                start=(k == 0),
                stop=(k == NK - 1),
            )
        nc.vector.tensor_copy(
            v8_blocks[:, jt, :, 0:D],
            p[:].rearrange("p (h e) -> p h e", e=D),
        )

    pt_tiles = {}

    def scores_exp(h, jts):
        """scoresT (j, i) for head h + exp(s - 3) -> pT fp8e4."""
        pr, hh = divmod(h, 2)
        if h in pt_tiles:
            pT = pt_tiles[h]
        else:
            pT = sbuf_pt_pool.tile([128, NJT * N], F8, tag="pt", name=f"pt{h}")
            pt_tiles[h] = pT
        po = 64 * hh
        for jt in jts:
            sp = ps.tile([128, N], F32, tag="score", bufs=2)
            for c in range(NCH):
                nc.tensor.matmul(
                    sp[:, 512 * c : 512 * c + 512],
                    kT_sb[po : po + 64, N * pr + 128 * jt : N * pr + 128 * jt + 128],
                    qT_sb[po : po + 64, N * pr + 512 * c : N * pr + 512 * c + 512],
                    start=True,
                    stop=True,
                    tile_position=(po, 0),
                )
            nc.scalar.activation(
                pT[:, N * jt : N * jt + N], sp[:], AF.Exp, bias=expb_sb[:]
            )

    pv_tiles = {}

    def attnv_unit(h, c, g2):
        """2 accumulating fp8 matmuls (j-tiles 2*g2, 2*g2+1) of outT~ for
        (h, c); denominator + os8 scaling after the last unit of the chunk."""
        pr, hh = divmod(h, 2)
        pT = pt_tiles[h]
        key = (h, c)
        if key not in pv_tiles:
            pv_tiles[key] = ps.tile([128, 512], F32, tag="mm", name=f"av{h}_{c}")
        p = pv_tiles[key]
        if USE_DR_ATTNV:
            lhs = v8_sb[:].rearrange(
                "p (jp two h e) -> p jp two h e", jp=NJT // 2, two=2, e=VR
            )[:, g2, :, h, 0 : D + 1]
            rhs = pT[:].rearrange(
                "p (jp two i) -> p jp two i", jp=NJT // 2, two=2
            )[:, g2, :, 512 * c : 512 * c + 512]
            nc.tensor.matmul(
                p[0:65, :],
                lhs,
                rhs,
                start=(g2 == 0),
                stop=(g2 == 3),
                perf_mode=PM.DoubleRow,
            )
        else:
            for jt in (2 * g2, 2 * g2 + 1):
                nc.tensor.matmul(
                    p[0:65, :],
                    v8_blocks[:, jt, h, 0 : D + 1],
                    pT[:, N * jt + 512 * c : N * jt + 512 * c + 512],
                    start=(jt == 0),
                    stop=(jt == NJT - 1),
                )
        if g2 == 3:
            del pv_tiles[key]
            if c == NCH - 1:
                del pt_tiles[h]
            r = 2 * h + c
            # denominator chain, DMA-free: one reciprocal straight off the
            # PSUM denom row (p64) writing partition 0, gpsimd broadcast,
            # then one fp32 multiply straight off PSUM into os8 (fp8).
            rrow = rr_pool.tile([1, 512], F32, tag="rrow", name=f"rr{r}")
            nc.vector.reciprocal(rrow[0:1, :], p[64:65, :])
            rb = rb_pool.tile([128, 512], F32, tag="rb", name=f"rb{r}")
            nc.gpsimd.partition_broadcast(rb[:], rrow[0:1, :])
            sl = slice(N * pr + 512 * c, N * pr + 512 * c + 512)
            nc.vector.tensor_tensor(
                os_sb[64 * hh : 64 * hh + 64, sl],
                p[0:64, :],
                rb[0:64, :],
                ALU.mult,
            )

    def outproj(ct):
        """yT c-tile ct: wl8.T @ os8 (fp8) + (x + b'), fp32 out + DMA.

        PSUM comes from the score tag's banks (idle once exps are done)."""
        p = ps.tile([128, N], F32, tag="score", bufs=2, name=f"yp{ct}")
        for c in range(NCH):
            if USE_DR_OUTPROJ:
                wl3 = wl8_sb[:].rearrange("p (up two km) -> p up two km", up=2, two=2)
                os3 = os_sb[:].rearrange("p (up two i) -> p up two i", up=2, two=2)
                for up in range(2):
                    nc.tensor.matmul(
                        p[:, 512 * c : 512 * c + 512],
                        wl3[:, up, :, 128 * ct : 128 * ct + 128],
                        os3[:, up, :, 512 * c : 512 * c + 512],
                        start=(up == 0),
                        stop=(up == 1),
                        perf_mode=PM.DoubleRow,
                    )
            else:
                for k in range(NK):
                    nc.tensor.matmul(
                        p[:, 512 * c : 512 * c + 512],
                        wl8_sb[:, 512 * k + 128 * ct : 512 * k + 128 * ct + 128],
                        os_sb[:, N * k + 512 * c : N * k + 512 * c + 512],
                        start=(k == 0),
                        stop=(k == NK - 1),
                    )
        if ct < 3:
            y = y_pool.tile([128, N], F32, tag="y")
            nc.vector.tensor_tensor(
                y[:], p[:], xr_sb[:, N * ct : N * ct + N], ALU.add
            )
            nc.sync.dma_start(d["y"].ap()[128 * ct : 128 * ct + 128, :], y[:])
        else:
            # split the last c-tile so its evac/DMA pipeline drains earlier
            for c in range(NCH):
                sl = slice(512 * c, 512 * c + 512)
                y = y_pool.tile([128, 512], F32, tag="y2", name=f"y3_{c}")
                nc.vector.tensor_tensor(
                    y[:], p[:, sl], xr_sb[:, N * ct + 512 * c : N * ct + 512 * c + 512],
                    ALU.add,
                )
                nc.sync.dma_start(
                    d["y"].ap()[128 * ct : 128 * ct + 128, sl], y[:]
                )

    # --- pools that emitters close over ---
    import contextlib

    stack = contextlib.ExitStack()
    sbuf_pt_pool = stack.enter_context(tc.tile_pool(name="pt", bufs=3))
    rb_pool = stack.enter_context(tc.tile_pool(name="rb", bufs=3))
    rr_pool = stack.enter_context(tc.tile_pool(name="rr", bufs=3))
    y_pool = stack.enter_context(tc.tile_pool(name="y", bufs=3))

    # --- software-pipelined emission (PE order: keep feeding while ACT exps) ---
    av_units = [(h, c, g2) for h in range(H) for c in range(NCH) for g2 in range(4)]
    av_pos = 0

    def drain_av(n):
        nonlocal av_pos
        for _ in range(n):
            if av_pos >= len(av_units):
                return
            h, c, g2 = av_units[av_pos]
            av_pos += 1
            attnv_unit(h, c, g2)

    qk_tile(0)
    qk_tile(1)
    qk_quarters = [
        (wsb, dst, bcol, t, c)
        for t in (2, 3)
        for (wsb, dst, bcol) in ((wq_sb, qT_sb, t), (wk_sb, kT_sb, 4 + t))
        for c in range(NCH)
    ]
    for g in range(NJT):  # head 0 scores + qk tiles 2,3 (one quarter per step)
        scores_exp(0, [g])
        wsb, dst, bcol, t, c = qk_quarters[g]
        p = ps.tile([128, 512], F32, tag="mm", name=f"qk{t}_{bcol}_{c}")
        for k in range(NK):
            nc.tensor.matmul(
                p[:],
                wsb[:, 512 * k + 128 * t : 512 * k + 128 * t + 128],
                x16_sb[:, N * k + 512 * c : N * k + 512 * c + 512],
                start=(k == 0),
                stop=(k == NK - 1),
            )
        nc.vector.tensor_scalar_add(
            dst[:, N * t + 512 * c : N * t + 512 * c + 512],
            p[:],
            bqk_sb[:, bcol : bcol + 1],
        )
    for g in range(NJT):  # head 1 scores + v tiles
        scores_exp(1, [g])
        v_tile(g)
    rates = {2: 8, 3: 8, 4: 8, 5: 8, 6: 8, 7: 8}
    for h in range(2, H):
        per = [rates[h] // NJT + (1 if g < rates[h] % NJT else 0) for g in range(NJT)]
        for g in range(NJT):
            scores_exp(h, [g])
            drain_av(per[g])
    drain_av(len(av_units))  # remainder (attnv of heads 6,7 + last denoms)
    for ct in range(4):
        outproj(ct)

    stack.close()


def _build(loop=1):
    nc = bacc.Bacc("TRN2", target_bir_lowering=False, debug=False, num_devices=BS)
    d = {}
    d["x16"] = nc.dram_tensor("x16", [CIN, N], F16, kind="ExternalInput")
    d["xr"] = nc.dram_tensor("xr", [CIN, N], F32, kind="ExternalInput")
    d["wq"] = nc.dram_tensor("wq", [CIN, 512], F16, kind="ExternalInput")
    d["wk"] = nc.dram_tensor("wk", [CIN, 512], F16, kind="ExternalInput")
    d["wv"] = nc.dram_tensor("wv", [CIN, 512], F16, kind="ExternalInput")
    d["wl8"] = nc.dram_tensor("wl8", [CIN, 512], F8, kind="ExternalInput")
    d["bqk"] = nc.dram_tensor("bqk", [128, 8], F32, kind="ExternalInput")
    d["y"] = nc.dram_tensor("y", [CIN, N], F32, kind="ExternalOutput")

    with tile.TileContext(nc) as tc:
        with (
            tc.tile_pool(name="sb", bufs=1) as sb,
            tc.tile_pool(name="ps", bufs=4, space="PSUM") as ps,
        ):
            for i in range(loop):
                if i:
                    with tc.tile_critical():
                        nc.all_engine_barrier()
                _emit(tc, d, sb, ps)
    nc.compile()
    return nc


_NC_CACHE = {}


def get_nc(loop=1):
    key = (loop, USE_DR_ATTNV, USE_DR_OUTPROJ)
    if key not in _NC_CACHE:
        _NC_CACHE[key] = _build(loop)
    return _NC_CACHE[key]


def host_prep(x, W_fc, b_fc, W_last, b_last):
    """Full inputs -> list of 8 per-core input maps."""
    x = np.asarray(x, dtype=np.float64)
    W_fc = np.asarray(W_fc, dtype=np.float64)
    b_fc = np.asarray(b_fc, dtype=np.float64)
    W_last = np.asarray(W_last, dtype=np.float64)
    b_last = np.asarray(b_last, dtype=np.float64)

    hh = np.arange(H).repeat(D) * 3 * D  # 192h per f'=64h+d
    dd = np.tile(np.arange(D), H)
    pq, pk, pv = hh + dd, hh + D + dd, hh + 2 * D + dd

    wq = np.ascontiguousarray((W_fc[pq] * 0.125).T).astype(np.float16)
    wk = np.ascontiguousarray(W_fc[pk].T).astype(np.float16)
    wv = np.ascontiguousarray(W_fc[pv].T).astype(np.float16)
    wl8 = np.ascontiguousarray(W_last.T).astype(ml_dtypes.float8_e4m3)
    bq, bk, bv = b_fc[pq] * 0.125, b_fc[pk], b_fc[pv]
    bqk = np.ascontiguousarray(
        np.concatenate([bq.reshape(4, 128).T, bk.reshape(4, 128).T], axis=1)
    ).astype(np.float32)
    # b_v passes through softmax unchanged (weights sum to 1): fold W_last@bv
    # into the residual bias.
    b_eff = b_last + W_last @ bv

    xf = x.reshape(BS, CIN, N)
    maps = []
    for b in range(BS):
        maps.append(
            {
                "x16": xf[b].astype(np.float16),
                "xr": (xf[b] + b_eff[:, None]).astype(np.float32),
                "wq": wq,
                "wk": wk,
                "wv": wv,
                "wl8": wl8,
                "bqk": bqk,
            }
        )
    return maps


def kernel(x, W_fc, b_fc, W_last, b_last):
    nc = get_nc()
    maps = host_prep(x, W_fc, b_fc, W_last, b_last)
    res = run_bass_kernel_spmd(nc, maps, core_ids=list(range(BS)))
    y = np.stack([res.results[b]["y"] for b in range(BS)])
    return y.reshape(BS, CIN, 32, 32)
